# revision 24
# baseline (speedup 1.0000x reference)
"""HNHN hypergraph GNN forward on 8 Trainium2 NeuronCores (Bass/Tile).

Sharding: edges 50k/core, nodes 25k/core (edge ids relabeled e -> (e%8, e//8)
for load balance; relabeling is internal, the output is a node max-pool).
Each segment aggregation is computed as PE matmuls
    psum[feat, 512segs] += G_block^T @ S_block
with G_block = 128 gathered bf16 source rows and S_block a one-hot x weight
selection matrix built on DVE via tensor_scalar(is_equal, mult) against an
iota tile. Layer-1 edge aggregation consumes a host-expanded per-nnz stream
of x_0 (input resharding; no gather). The other three aggregations gather
device-computed bf16 tables with dma_gather (1024 rows/instruction, int16
indices => 25088-row rank-aligned buckets; nnz sorted by (psum-section,
bucket, seg); runs padded with bucket-row-0 slots carrying weight 0). The
selection matrix is built once per 128-slot block in section-relative
coordinates (one DVE tensor_scalar), and per-window matmuls slice it.
Tables are AllGather'd in 2 rank-major chunk tensors so each chunk fires as
soon as its producing window flushes complete and each gather bucket
depends on exactly one chunk. Flush psum->SBUF copies run on the scalar
(ACT) engine to keep DVE on selection-matrix builds. The per-layer dense
matmul, sigmoid (+per-partition bias) happen in the transposed [feat, seg]
domain; PE transposes restore row-major bf16 tables. Final: running window
max, AllReduce(max), f32 dot with lin_w.

Perf notes (measured on axon-tunneled trn2, drift-controlled A/B): the
bottleneck is the GPSIMD Q7 SWDGE descriptor path, ~8-10 ns per gathered
row (675k rows/core across the 3 gathered aggregations ~= 5 ms); DVE/PE
job pipeline ~2-3 ms overlaps it. dma_gather cost is per-descriptor, not
per-byte, and insensitive to index locality; ap_gather ucode is ~3x worse.
"""

import numpy as np
import ml_dtypes

bf16 = ml_dtypes.bfloat16
f32 = np.float32

import os as _os_top
P = 128
WIN = 512          # segments per PSUM window (one bank)
SECW = int(_os_top.environ.get("SECW", "5"))
                   # windows/section; PSUM: SECW win + 1 tp + PM m = 8 banks
BUCK = int(_os_top.environ.get("BUCK", "25088"))
                   # bucket rows: rank-aligned (= node_pad = edge_pad/2) so a
                   # gather bucket depends on exactly one AllGather chunk;
                   # must stay < 32768 (int16 dma_gather indices)
NIMAX = int(_os_top.environ.get("NIMAX", "1024"))  # idxs per dma_gather inst
NCORES = 8

N_NODES = 200_000
N_EDGES = 400_000
IN_CH = 14
HID = 128
ALPHA = -1.5
BETA = -0.5


def _dims():
    node_loc = N_NODES // NCORES
    edge_loc = N_EDGES // NCORES
    node_wins = -(-node_loc // WIN)
    edge_wins = -(-edge_loc // WIN)
    return dict(
        node_loc=node_loc, edge_loc=edge_loc,
        node_pad=node_wins * WIN, edge_pad=edge_wins * WIN,
        node_tab=node_wins * WIN * NCORES, edge_tab=edge_wins * WIN * NCORES,
    )


def _wrap_idx(flat):
    a = flat.reshape(-1, 16).T.astype(np.int16)
    return np.tile(a, (8, 1))


class AggSched:
    """Schedule + per-core metadata for one aggregation (SPMD-identical)."""

    def __init__(self, name, dest_loc, src_all, w_all, n_seg_loc, tab_rows,
                 gathered):
        self.name = name
        self.gathered = gathered
        self.n_seg_loc = n_seg_loc
        self.n_wins = -(-n_seg_loc // WIN)
        sec = WIN * SECW
        self.n_secs = -(-self.n_wins // SECW)
        nbuck = -(-tab_rows // BUCK) if gathered else 1

        per_core = []
        counts = np.zeros((NCORES, self.n_secs, nbuck), np.int64)
        for r in range(NCORES):
            d = dest_loc[r].astype(np.int64)
            s = src_all[r].astype(np.int64)
            w = w_all[r].astype(f32)
            sc = d // sec
            b = (s // BUCK) if gathered else np.zeros_like(s)
            order = np.lexsort((d, b, sc))
            d, s, w, sc, b = d[order], s[order], w[order], sc[order], b[order]
            per_core.append((d, s, w, sc, b))
            np.add.at(counts[r], (sc, b), 1)

        self.runs = []          # (sec, bucket, n_pad_slots)
        for sc in range(self.n_secs):
            for b in range(nbuck):
                c = counts[:, sc, b].max()
                if c:
                    self.runs.append((sc, b, int(-(-c // 128) * 128)))
        total_slots = sum(np_ for _, _, np_ in self.runs)
        self.n_blocks = total_slots // 128

        dmat = np.full((NCORES, total_slots), -1, np.int64)
        smat = np.full((NCORES, total_slots), -1, np.int64)
        wmat = np.zeros((NCORES, total_slots), f32)
        for r in range(NCORES):
            d, s, w, sc, b = per_core[r]
            off = 0
            ptr = 0
            for rsec, rb, n_pad in self.runs:
                cnt = int(counts[r, rsec, rb])
                dmat[r, off:off + cnt] = d[ptr:ptr + cnt]
                smat[r, off:off + cnt] = (s[ptr:ptr + cnt] % BUCK) if gathered \
                    else s[ptr:ptr + cnt]
                if gathered:
                    smat[r, off + cnt:off + n_pad] = 0  # pad -> bucket row 0
                wmat[r, off:off + cnt] = w[ptr:ptr + cnt]
                ptr += cnt
                off += n_pad
            assert ptr == len(d)

        self.insts = []         # (bucket, slot_off, ni)
        off = 0
        for sc, b, n_pad in self.runs:
            o = 0
            while o < n_pad:
                ni = min(NIMAX, n_pad - o)
                self.insts.append((b, off + o, ni))
                o += ni
            off += n_pad

        # Per-block selection matrix in SECTION-relative coordinates: one
        # tensor_scalar build per block; per-window matmuls slice it.
        sec_w = WIN * SECW
        blk_sec = np.empty(self.n_blocks, np.int64)
        off = 0
        for rsec, rb, n_pad in self.runs:
            blk_sec[off // 128:(off + n_pad) // 128] = rsec
            off += n_pad
        dblk = dmat.reshape(NCORES, self.n_blocks, 128)
        self.blocks = []        # (block, c0, c1) in section-relative coords
        self.jobs = []          # (block_idx, window, mm_a, mm_b)  [win-rel]
        self.win_last = {}
        for blk in range(self.n_blocks):
            dv = dblk[:, blk, :]
            valid = dv >= 0
            if not valid.any():
                continue
            sec = int(blk_sec[blk])
            dsec = dv - sec * sec_w
            c0 = int(dsec[valid].min())
            c1 = int(dsec[valid].max()) + 1
            bi = len(self.blocks)
            self.blocks.append((blk, c0, c1))
            for wname in np.unique(dv[valid] // WIN):
                wrel0 = (int(wname) - sec * SECW) * WIN
                m = valid & (dv // WIN == wname)
                a = int(dsec[m].min())
                b = int(dsec[m].max()) + 1
                # out slice [o0:o1) window-relative; rhs slice [r0:r1)
                # relative to the block's s_t (built over [c0, c1))
                self.jobs.append((bi, int(wname), a - wrel0, b - wrel0,
                                  a - c0, b - c0))
                self.win_last[int(wname)] = len(self.jobs) - 1
        self.n_jobs = len(self.jobs)
        self.n_sblocks = len(self.blocks)

        self.w_arr = np.ascontiguousarray(
            wmat.reshape(NCORES, self.n_blocks, 128).transpose(0, 2, 1))
        self.seg_arr = np.full((NCORES, P, max(self.n_sblocks, 1)), -1.0, f32)
        for bi, (blk, c0, c1) in enumerate(self.blocks):
            sec = int(blk_sec[blk])
            dv = dblk[:, blk, :]                    # [NCORES, 128]
            ok = dv >= 0
            self.seg_arr[:, :, bi] = np.where(ok, dv - sec * sec_w, -1.0)
        self.seg_arr = self.seg_arr.astype(f32)

        if gathered:
            idx_cols = sum(ni for _, _, ni in self.insts) // 16
            self.idx_arr = np.zeros((NCORES, P, idx_cols), np.int16)
            self.inst_idx_off = []
            col = 0
            for b, slot_off, ni in self.insts:
                self.inst_idx_off.append(col)
                for r in range(NCORES):
                    self.idx_arr[r, :, col:col + ni // 16] = _wrap_idx(
                        smat[r, slot_off:slot_off + ni].astype(np.int16))
                col += ni // 16
            self.idx_cols = idx_cols
        else:
            self.smat = smat


def _preprocess(inputs):
    dims = _dims()
    rows = np.asarray(inputs["inc_rows"]).astype(np.int64)
    cols0 = np.asarray(inputs["inc_cols"]).astype(np.int64)
    vals = np.asarray(inputs["inc_vals"]).astype(f32)

    # relabel edges for per-core balance: e -> (e % NCORES)*edge_loc + e//NCORES
    cols = (cols0 % NCORES) * dims["edge_loc"] + cols0 // NCORES

    deg_e = np.bincount(cols, weights=vals, minlength=N_EDGES).astype(f32)
    deg_v = np.bincount(rows, weights=vals, minlength=N_NODES).astype(f32)
    e_card = deg_e ** f32(ALPHA)
    n_card = deg_v ** f32(BETA)
    denom_v = np.bincount(rows, weights=(vals * e_card[cols]).astype(np.float64),
                          minlength=N_NODES).astype(f32)
    denom_e = np.bincount(cols, weights=(vals * n_card[rows]).astype(np.float64),
                          minlength=N_EDGES).astype(f32)
    w_ev = vals * n_card[rows] / denom_e[cols]
    w_ve = vals * e_card[cols] / denom_v[rows]

    e_core = cols // dims["edge_loc"]
    v_core = rows // dims["node_loc"]
    node_pad_row = rows // dims["node_loc"] * dims["node_pad"] \
        + rows % dims["node_loc"]
    edge_pad_row = cols // dims["edge_loc"] * dims["edge_pad"] \
        + cols % dims["edge_loc"]

    def split(arr, by):
        return [arr[by == r] for r in range(NCORES)]

    e_d = split(cols % dims["edge_loc"], e_core)
    e_s_raw = split(rows, e_core)
    e_s_pad = split(node_pad_row, e_core)
    e_w = split(w_ev, e_core)
    n_d = split(rows % dims["node_loc"], v_core)
    n_s = split(edge_pad_row, v_core)
    n_w = split(w_ve, v_core)

    sched_e1 = AggSched("e1", e_d, e_s_raw, e_w, dims["edge_loc"], N_NODES,
                        False)
    sched_e2 = AggSched("e2", e_d, e_s_pad, e_w, dims["edge_loc"],
                        dims["node_tab"], True)
    sched_n = AggSched("n", n_d, n_s, n_w, dims["node_loc"],
                       dims["edge_tab"], True)

    x0 = np.asarray(inputs["x_0"]).astype(f32)
    x0p = np.zeros((N_NODES + 1, 16), f32)
    x0p[:N_NODES, :IN_CH] = x0
    e1_stream = np.zeros((NCORES, P, sched_e1.n_blocks * 16), bf16)
    for r in range(NCORES):
        src = sched_e1.smat[r].reshape(sched_e1.n_blocks, 128)
        g = x0p[np.where(src >= 0, src, N_NODES)]
        e1_stream[r] = g.transpose(1, 0, 2).reshape(P, -1).astype(bf16)

    return dict(sched_e1=sched_e1, sched_e2=sched_e2, sched_n=sched_n,
                e1_stream=e1_stream, dims=dims)


def _build(pre):
    import concourse.bacc as bacc
    import concourse.mybir as mybir
    import concourse.tile as tile

    dt = mybir.dt
    dims = pre["dims"]
    import os as _osb
    nc = bacc.Bacc("TRN2", target_bir_lowering=False, debug=False,
                   num_devices=NCORES,
                   dynamic_dma_scratch_size=int(
                       _osb.environ.get("DDS", "65536")))

    s_e1, s_e2, s_n = pre["sched_e1"], pre["sched_e2"], pre["sched_n"]

    def din(name, shape, dtyp):
        return nc.dram_tensor(name, shape, dtyp, kind="ExternalInput")

    e1_g = din("e1_g", [P, s_e1.n_blocks * 16], dt.bfloat16)
    e1_seg = din("e1_seg", [P, max(s_e1.n_sblocks, 1)], dt.float32)
    e1_w = din("e1_w", [P, s_e1.n_blocks], dt.float32)
    n1_idx = din("n1_idx", [P, s_n.idx_cols], dt.int16)
    n1_seg = din("n1_seg", [P, max(s_n.n_sblocks, 1)], dt.float32)
    n1_w = din("n1_w", [P, s_n.n_blocks], dt.float32)
    e2_idx = din("e2_idx", [P, s_e2.idx_cols], dt.int16)
    e2_seg = din("e2_seg", [P, max(s_e2.n_sblocks, 1)], dt.float32)
    e2_w = din("e2_w", [P, s_e2.n_blocks], dt.float32)

    w_in = {k: din(k, [kd, HID], dt.bfloat16)
            for k, kd in (("w0_1", 16), ("w1_1", HID), ("w0_2", HID),
                          ("w1_2", HID))}
    b_in = {k: din(k, [P, 1], dt.float32)
            for k in ("b1_1", "b0_1", "b1_2", "b0_2")}
    lin_w = din("lin_w", [P, 1], dt.float32)
    lin_b = din("lin_b", [1, 1], dt.float32)
    iota_in = din("iota", [P, WIN * SECW], dt.float32)
    ident_in = din("ident", [P, P], dt.bfloat16)
    out_t = nc.dram_tensor("out", [1, 1], dt.float32, kind="ExternalOutput")

    def dint(name, shape, shared=False):
        return nc.dram_tensor(name, shape, dt.bfloat16, kind="Internal",
                              addr_space="Shared" if shared else "Local")

    import os as _os1
    AGCH = int(_os1.environ.get("AGCH", "2"))
    # Edge tables are AllGather'd in AGCH chunks, one contiguous Shared
    # tensor per chunk (rank-major within the chunk). A gather bucket
    # (BUCK = edge_pad/AGCH rows, rank-aligned) then lives in exactly one
    # chunk tensor, so its dma_gather depends only on that chunk's AG.
    if AGCH > 1:
        ech = dims["edge_pad"] // AGCH
        assert ech * AGCH == dims["edge_pad"] and ech == BUCK
    else:
        ech = dims["edge_pad"]
    x1l1_loc = dint("x1l1_loc", [dims["edge_pad"], HID])
    x1l1_full = [dint(f"x1l1_full{h}", [NCORES * ech, HID], True)
                 for h in range(AGCH)]
    x0p_loc = dint("x0p_loc", [dims["node_pad"], HID])
    x0p_full = dint("x0p_full", [dims["node_tab"], HID], True)
    x1l2_loc = dint("x1l2_loc", [dims["edge_pad"], HID])
    x1l2_full = [dint(f"x1l2_full{h}", [NCORES * ech, HID], True)
                 for h in range(AGCH)]
    armax_in = nc.dram_tensor("armax_in", [P, 1], dt.float32, kind="Internal")
    armax_out = nc.dram_tensor("armax_out", [P, 1], dt.float32,
                               kind="Internal", addr_space="Shared")

    import os as _os0
    GP_BUFS = int(_os0.environ.get("GP_BUFS", "8"))
    SP_BUFS = int(_os0.environ.get("SP_BUFS", "6"))
    with tile.TileContext(nc) as tc:
        with tc.tile_pool(name="const", bufs=1) as cp, \
             tc.tile_pool(name="meta", bufs=int(_os0.environ.get("MP_BUFS", "1"))) as mp, \
             tc.tile_pool(name="gt", bufs=GP_BUFS) as gp, \
             tc.tile_pool(name="st", bufs=SP_BUFS) as sp, \
             tc.tile_pool(name="fl", bufs=2) as fp, \
             tc.tile_pool(name="psw", bufs=1, space="PSUM") as pw, \
             tc.tile_pool(name="psm", bufs=(2 if SECW <= 5 else 1), space="PSUM") as pm:

            iota_t = cp.tile([P, WIN * SECW], dt.float32)
            ident_t = cp.tile([P, P], dt.bfloat16)
            nc.sync.dma_start(iota_t[:], iota_in[:])
            nc.sync.dma_start(ident_t[:], ident_in[:])
            wts, bias = {}, {}
            for k, hnd in w_in.items():
                t = cp.tile(list(hnd.shape), dt.bfloat16, tag=k)
                nc.sync.dma_start(t[:], hnd[:])
                wts[k] = t
            for k, hnd in b_in.items():
                t = cp.tile([P, 1], dt.float32, tag=k)
                nc.sync.dma_start(t[:], hnd[:])
                bias[k] = t
            linw_t = cp.tile([P, 1], dt.float32)
            nc.sync.dma_start(linw_t[:], lin_w[:])
            linb_t = cp.tile([1, 1], dt.float32)
            nc.sync.dma_start(linb_t[:], lin_b[:])
            maxacc = cp.tile([P, WIN], dt.bfloat16)
            nc.vector.memset(maxacc[:], -1.0)

            def run_agg(sched, seg_d, w_d, kdim, wkey, bkey, tables, out_loc,
                        idx_d=None, stream_d=None, maxpool=False):
                import os as _os
                LHSW = int(_os.environ.get("LHSW", "0"))
                if LHSW and sched.gathered:
                    kdim = LHSW
                seg_t = mp.tile([P, max(sched.n_sblocks, 1)], dt.float32,
                                tag="seg")
                w_t = mp.tile([P, sched.n_blocks], dt.float32, tag="w")
                nc.sync.dma_start(seg_t[:], seg_d[:])
                nc.sync.dma_start(w_t[:], w_d[:])

                import os as _os
                NOGATH = int(_os.environ.get("NOGATH", "0"))
                NOJOBS = int(_os.environ.get("NOJOBS", "0"))
                blk_slice = {}
                if sched.gathered:
                    idx_t = mp.tile([P, sched.idx_cols], dt.int16, tag="idx")
                    nc.sync.dma_start(idx_t[:], idx_d[:])
                    nch = len(tables)
                    SPKT = bool(int(_os.environ.get("SPKT", "0")))
                    for k, (b, slot_off, ni) in enumerate(sched.insts):
                        g = gp.tile([P, (NIMAX // P) * HID], dt.bfloat16,
                                    tag="g")
                        off = sched.inst_idx_off[k]
                        tbl = tables[b % nch]
                        roff = (b // nch) * BUCK
                        tab_rows = tbl.shape[0]
                        if not NOGATH:
                            nc.gpsimd.dma_gather(
                                g[:, :(ni // P) * HID].rearrange(
                                    "p (n f) -> p n f", f=HID),
                                tbl[roff:min(roff + BUCK, tab_rows), :],
                                idx_t[:, off:off + ni // 16],
                                ni, ni, HID, single_packet=SPKT)
                        else:
                            nc.vector.memset(g[:1, :1], 0.0)
                        for cb in range(ni // P):
                            blk_slice[slot_off // P + cb] = (g, cb * HID, HID)
                else:
                    SLAB = 32
                    for sl in range(-(-sched.n_blocks // SLAB)):
                        b0 = sl * SLAB
                        nb = min(SLAB, sched.n_blocks - b0)
                        g = gp.tile([P, SLAB * 16], dt.bfloat16, tag="g")
                        nc.sync.dma_start(g[:, :nb * 16],
                                          stream_d[:, b0 * 16:(b0 + nb) * 16])
                        for cb in range(nb):
                            blk_slice[b0 + cb] = (g, cb * 16, 16)

                win_tiles = {}

                FLUSH_ACT = int(_os.environ.get("FLUSH_ACT", "1"))

                def flush(wn):
                    psum1 = win_tiles.pop(wn)
                    aggt = fp.tile([kdim, WIN], dt.bfloat16, tag="aggt")
                    if FLUSH_ACT:
                        nc.scalar.copy(aggt[:], psum1[:])
                    else:
                        nc.vector.tensor_copy(aggt[:], psum1[:])
                    psum2 = pm.tile([P, WIN], dt.float32, tag="m",
                                    space="PSUM")
                    nc.tensor.matmul(psum2[:], lhsT=wts[wkey][:kdim, :],
                                     rhs=aggt[:], start=True, stop=True)
                    xt = fp.tile([P, WIN], dt.bfloat16, tag="xt")
                    nc.scalar.activation(xt[:], psum2[:],
                                         mybir.ActivationFunctionType.Sigmoid,
                                         bias=bias[bkey][:, :1], scale=1.0)
                    if maxpool:
                        nv = min(WIN, sched.n_seg_loc - wn * WIN)
                        nc.vector.tensor_tensor(
                            out=maxacc[:, :nv], in0=maxacc[:, :nv],
                            in1=xt[:, :nv], op=mybir.AluOpType.max)
                    else:
                        nq = WIN // P
                        rowt = fp.tile([P, WIN], dt.bfloat16, tag="rowt")
                        for q in range(nq):
                            pt = pw.tile([P, P], dt.bfloat16, tag="tp",
                                         space="PSUM")
                            nc.tensor.transpose(pt[:],
                                                xt[:, q * P:(q + 1) * P],
                                                ident_t[:])
                            if FLUSH_ACT:
                                nc.scalar.copy(rowt[:, q * P:(q + 1) * P],
                                               pt[:])
                            else:
                                nc.vector.tensor_copy(
                                    rowt[:, q * P:(q + 1) * P], pt[:])
                        nc.sync.dma_start(
                            out_loc[wn * WIN:(wn + 1) * WIN, :].rearrange(
                                "(q p) f -> p q f", p=P),
                            rowt[:].rearrange("p (q f) -> p q f", q=nq))

                # Per-block selection-matrix build (section-relative iota),
                # then one matmul per (block, window) slicing s_t.
                jobs = sched.jobs
                jp = 0
                for bi, (blk, c0, c1) in enumerate(sched.blocks):
                    if NOJOBS:
                        break
                    g, goff, gw = blk_slice[blk]
                    span = c1 - c0
                    s_t = sp.tile([P, WIN * SECW], dt.bfloat16, tag="s")
                    nc.vector.tensor_scalar(
                        out=s_t[:, :span], in0=iota_t[:, c0:c1],
                        scalar1=seg_t[:, bi:bi + 1],
                        scalar2=w_t[:, blk:blk + 1],
                        op0=mybir.AluOpType.is_equal,
                        op1=mybir.AluOpType.mult)
                    MEMS_ACT = int(_os.environ.get("MEMS_ACT", "1"))
                    while jp < len(jobs) and jobs[jp][0] == bi:
                        _, wn, o0, o1, r0, r1 = jobs[jp]
                        if wn not in win_tiles:
                            pt = pw.tile([kdim, WIN], dt.float32,
                                         tag=f"win{wn % SECW}", space="PSUM")
                            if MEMS_ACT:
                                nc.scalar.memzero(pt[:])
                            else:
                                nc.vector.memset(pt[:], 0.0)
                            win_tiles[wn] = pt
                        nc.tensor.matmul(
                            win_tiles[wn][:, o0:o1],
                            lhsT=g[:, goff:goff + kdim],
                            rhs=s_t[:, r0:r1], start=False,
                            stop=(sched.win_last[wn] == jp),
                            skip_group_check=True)
                        if sched.win_last[wn] == jp:
                            flush(wn)
                        jp += 1

            import os
            PH = int(os.environ.get("PHASES", "4"))
            NOCOLL = int(os.environ.get("NOCOLL", "0"))
            rg = [list(range(NCORES))]

            def ag_chunked(loc, fulls, n_rows):
                # Chunked AllGather: chunk h covers local rows
                # [h*n_rows/nch, ...) into its own contiguous Shared tensor
                # (rank-major inside the chunk). Each chunk fires as soon as
                # the producing flushes of those rows complete (mid-phase),
                # and a gather bucket reads exactly one chunk tensor, so
                # early buckets start before the whole table is in.
                if NOCOLL:
                    return
                nch = len(fulls)
                step = n_rows // nch
                for h in range(nch):
                    nc.gpsimd.collective_compute(
                        "AllGather", mybir.AluOpType.bypass,
                        replica_groups=rg,
                        ins=[loc[h * step:(h + 1) * step, :]],
                        outs=[fulls[h][:]])

            if PH >= 1:
                run_agg(s_e1, e1_seg, e1_w, 16, "w0_1", "b1_1", None, x1l1_loc,
                        stream_d=e1_g)
            if PH >= 2:
                ag_chunked(x1l1_loc, x1l1_full, dims["edge_pad"])
                run_agg(s_n, n1_seg, n1_w, HID, "w1_1", "b0_1", x1l1_full,
                        x0p_loc, idx_d=n1_idx)
            if PH >= 3:
                ag_chunked(x0p_loc, [x0p_full], dims["node_pad"])
                run_agg(s_e2, e2_seg, e2_w, HID, "w0_2", "b1_2", [x0p_full],
                        x1l2_loc, idx_d=e2_idx)
            if PH >= 4:
                ag_chunked(x1l2_loc, x1l2_full, dims["edge_pad"])
                run_agg(s_n, n1_seg, n1_w, HID, "w1_2", "b0_2", x1l2_full,
                        None, idx_d=n1_idx, maxpool=True)

            mx = fp.tile([P, 1], dt.float32, tag="mx")
            nc.vector.reduce_max(out=mx[:], in_=maxacc[:],
                                 axis=mybir.AxisListType.X)
            nc.sync.dma_start(armax_in[:], mx[:])
            nc.gpsimd.collective_compute(
                "AllReduce", mybir.AluOpType.max, replica_groups=rg,
                ins=[armax_in[:]], outs=[armax_out[:]])
            mx2 = fp.tile([P, 1], dt.float32, tag="mx2")
            nc.sync.dma_start(mx2[:], armax_out[:])
            prod = fp.tile([P, 1], dt.float32, tag="prod")
            nc.vector.tensor_mul(prod[:], mx2[:], linw_t[:])
            ones = cp.tile([P, 1], dt.float32, tag="ones")
            nc.vector.memset(ones[:], 1.0)
            psf = pw.tile([1, 1], dt.float32, tag="tp", space="PSUM")
            nc.tensor.matmul(psf[:], lhsT=prod[:], rhs=ones[:],
                             start=True, stop=True)
            res = fp.tile([1, 1], dt.float32, tag="res")
            nc.scalar.activation(res[:], psf[:],
                                 mybir.ActivationFunctionType.Identity,
                                 bias=linb_t[:, :1], scale=1.0)
            nc.sync.dma_start(out_t[:], res[:])

    nc.compile()
    return nc


def make_in_maps(pre, inputs):
    s_e1, s_e2, s_n = pre["sched_e1"], pre["sched_e2"], pre["sched_n"]
    iota = np.broadcast_to(np.arange(WIN * SECW, dtype=f32),
                       (P, WIN * SECW)).copy()
    ident = np.eye(P, dtype=bf16)

    def b_t(x):
        return np.asarray(x).astype(f32).reshape(HID, 1)

    w0_1 = np.zeros((16, HID), bf16)
    w0_1[:IN_CH] = np.asarray(inputs["w0_l1"]).astype(bf16)
    in_maps = []
    for r in range(NCORES):
        in_maps.append(dict(
            e1_g=pre["e1_stream"][r],
            e1_seg=np.ascontiguousarray(s_e1.seg_arr[r]),
            e1_w=np.ascontiguousarray(s_e1.w_arr[r]),
            n1_idx=np.ascontiguousarray(s_n.idx_arr[r]),
            n1_seg=np.ascontiguousarray(s_n.seg_arr[r]),
            n1_w=np.ascontiguousarray(s_n.w_arr[r]),
            e2_idx=np.ascontiguousarray(s_e2.idx_arr[r]),
            e2_seg=np.ascontiguousarray(s_e2.seg_arr[r]),
            e2_w=np.ascontiguousarray(s_e2.w_arr[r]),
            w0_1=w0_1,
            w1_1=np.asarray(inputs["w1_l1"]).astype(bf16),
            w0_2=np.asarray(inputs["w0_l2"]).astype(bf16),
            w1_2=np.asarray(inputs["w1_l2"]).astype(bf16),
            b1_1=b_t(inputs["b1_l1"]), b0_1=b_t(inputs["b0_l1"]),
            b1_2=b_t(inputs["b1_l2"]), b0_2=b_t(inputs["b0_l2"]),
            lin_w=np.asarray(inputs["lin_w"]).astype(f32).reshape(HID, 1),
            lin_b=np.asarray(inputs["lin_b"]).astype(f32).reshape(1, 1),
            iota=iota, ident=ident,
        ))
    return in_maps


def kernel(**inputs):
    pre = _preprocess(inputs)
    nc = _build(pre)
    in_maps = make_in_maps(pre, inputs)
    from concourse.bass_utils import run_bass_kernel_spmd
    res = run_bass_kernel_spmd(nc, in_maps, core_ids=list(range(NCORES)))
    out = res.results[0]["out"].reshape(1).astype(f32)
    return out

